# revision 4
# baseline (speedup 1.0000x reference)
"""Multi-head attention (RoPE, non-causal) on 8 Trainium2 NeuronCores.

Problem: x[4,2048,2048] fp32; wq/wk/wv/wo [2048,2048]; biases [2048].
  q,k,v = x@w.T+b per 16 heads of dim 128; rope(q,k); softmax(q k^T/sqrt(128));
  out = (attn@v)@wo.T + bo.

Sharding: core c = 2*b + g -> batch b, head-group g (8 heads each).
Each core computes a partial output (its 8 heads) for its batch over the full
sequence; host sums the pair partials (the wo contraction splits over head
groups) and adds bo_eff = bo + wo@bv (the V-bias folds out exactly because
softmax rows sum to 1).

Device program (SPMD, one NEFF):
  P1: xT resident in SBUF. QT/KT per head = wT-chunk-stationary matmuls into
      PSUM (fp32r), ACT copy w/ bias (+1/sqrt(128) scale for Q), RoPE via
      DMA rotate-half + DVE mul/mul/add, spill to DRAM. V in natural [t,dh]
      layout (xT-chunk stationary), spilled to DRAM.
  P2: per head, flash-style over t-chunks: scoresT[t,s] matmul -> ACT exp ->
      ctxT accumulation in PSUM + DVE partial sums for the denominator;
      denominator finished with a ones-vector matmul (cross-partition sum),
      broadcast via partition-stride-0 DMA, DVE reciprocal+mul; ctxT -> DRAM.
      No max-subtraction: |scores| <= ~15 so exp is fp32-safe.
  P3: out[s,:] += ctxT_c^T @ woT_c over the core's 8 head-chunks.
"""

import sys

if "/opt/trn_rl_repo" not in sys.path:
    sys.path.insert(0, "/opt/trn_rl_repo")

import numpy as np

import concourse.bass as bass
import concourse.tile as tile
from concourse import bacc, mybir
from concourse.bass_utils import run_bass_kernel_spmd

F32 = mybir.dt.float32
F32R = mybir.dt.float32r

B, S, D = 4, 2048, 2048
H = 16
DH = 128
HL = 8  # heads per core
KO = D // 128  # 16 k-chunks
NB = S // 512  # 4 n-blocks of 512
TB = S // 128  # 16 t-chunks
ROPE_THETA = 10000.0
QSCALE = 1.0 / np.sqrt(DH)

_NC_CACHE = {}


def r32(ap):
    return ap.bitcast(F32R)


def build_nc():
    nc = bacc.Bacc()

    xt_d = nc.declare_dram_parameter("xt", [KO, 128, S], F32, isOutput=False)
    wq_d = nc.declare_dram_parameter("wq", [HL, KO, 128, 128], F32, isOutput=False)
    wk_d = nc.declare_dram_parameter("wk", [HL, KO, 128, 128], F32, isOutput=False)
    wv_d = nc.declare_dram_parameter("wv", [KO, 128, HL * DH], F32, isOutput=False)
    wo_d = nc.declare_dram_parameter("wo", [HL, 128, D], F32, isOutput=False)
    cos_d = nc.declare_dram_parameter("cosT", [128, S], F32, isOutput=False)
    sin_d = nc.declare_dram_parameter("sinS", [128, S], F32, isOutput=False)
    bq_d = nc.declare_dram_parameter("bq", [128, HL], F32, isOutput=False)
    bk_d = nc.declare_dram_parameter("bk", [128, HL], F32, isOutput=False)
    out_d = nc.declare_dram_parameter("out", [S, D], F32, isOutput=True)

    # DRAM spill for Q^T/K^T (per head, [dh, s]), natural-layout V, ctx^T
    qt_d = nc.dram_tensor("qt_spill", [HL, 128, S], F32)
    kt_d = nc.dram_tensor("kt_spill", [HL, 128, S], F32)
    v_d = nc.dram_tensor("v_spill", [TB, 128, HL * DH], F32)
    ct_d = nc.dram_tensor("ct_spill", [HL, 128, S], F32)
    den_d = nc.dram_tensor("den_bounce", [HL, 1, S], F32)

    with tile.TileContext(nc) as tc:
        # ---------------- Phase 1: projections ----------------
        with (
            tc.tile_pool(name="xt_pool", bufs=1) as xt_pool,
            tc.tile_pool(name="const_pool", bufs=1) as const_pool,
            tc.tile_pool(name="p1_psum", bufs=2, space="PSUM") as p1_psum,
        ):
            xt_sb = xt_pool.tile([128, KO, S], F32R)
            nc.sync.dma_start(
                out=xt_sb[:], in_=xt_d[:].rearrange("k p s -> p k s").bitcast(F32R)
            )

            cos_sb = const_pool.tile([128, S], F32)
            sin_sb = const_pool.tile([128, S], F32)
            bq_sb = const_pool.tile([128, HL], F32)
            bk_sb = const_pool.tile([128, HL], F32)
            nc.sync.dma_start(out=cos_sb[:], in_=cos_d[:])
            nc.sync.dma_start(out=sin_sb[:], in_=sin_d[:])
            nc.sync.dma_start(out=bq_sb[:], in_=bq_d[:])
            nc.sync.dma_start(out=bk_sb[:], in_=bk_d[:])

            # ---- V in natural layout [t, dh], all heads, two 512-col halves
            with (
                tc.tile_pool(name="wvh_pool", bufs=1) as wvh_pool,
                tc.tile_pool(name="vout_pool", bufs=3) as vout_pool,
            ):
                for hf in range(2):
                    wv_sb = wvh_pool.tile([128, KO, 512], F32R)
                    nc.sync.dma_start(
                        out=wv_sb[:],
                        in_=wv_d[:, :, hf * 512 : (hf + 1) * 512]
                        .rearrange("k p m -> p k m")
                        .bitcast(F32R),
                    )
                    for tb in range(TB):
                        vps = p1_psum.tile([128, 512], F32, tag="qk")
                        for ko in range(KO):
                            nc.tensor.matmul(
                                vps[:],
                                xt_sb[:, ko, tb * 128 : (tb + 1) * 128],
                                wv_sb[:, ko, :],
                                start=(ko == 0),
                                stop=(ko == KO - 1),
                            )
                        vsb = vout_pool.tile([128, 512], F32)
                        nc.scalar.copy(out=vsb[:], in_=vps[:])
                        nc.sync.dma_start(
                            out=v_d[tb, :, hf * 512 : (hf + 1) * 512], in_=vsb[:]
                        )

            # ---- Q^T / K^T per head with bias + rope
            with (
                tc.tile_pool(name="w_pool", bufs=2) as w_pool,
                tc.tile_pool(name="qk_pool", bufs=2) as qk_pool,
                tc.tile_pool(name="rot_pool", bufs=2) as rot_pool,
            ):
                for h in range(HL):
                    for w_d, b_sb, scale, dst in (
                        (wq_d, bq_sb, QSCALE, qt_d),
                        (wk_d, bk_sb, 1.0, kt_d),
                    ):
                        w_sb = w_pool.tile([128, KO, 128], F32R)
                        nc.sync.dma_start(
                            out=w_sb[:],
                            in_=w_d[h].rearrange("k p m -> p k m").bitcast(F32R),
                        )
                        qps = p1_psum.tile([128, S], F32, tag="qk")
                        for ko in range(KO):
                            for n in range(NB):
                                nc.tensor.matmul(
                                    qps[:, n * 512 : (n + 1) * 512],
                                    w_sb[:, ko, :],
                                    xt_sb[:, ko, n * 512 : (n + 1) * 512],
                                    start=(ko == 0),
                                    stop=(ko == KO - 1),
                                )
                        qs = qk_pool.tile([128, S], F32)
                        # qs = psum*scale + bias  (bias pre-scaled on host for Q)
                        nc.scalar.activation(
                            out=qs[:],
                            in_=qps[:],
                            func=mybir.ActivationFunctionType.Identity,
                            bias=b_sb[:, h : h + 1],
                            scale=scale,
                        )
                        # rope: q' = q*cos + rot(q)*sinS  (sinS sign-folded)
                        rot = rot_pool.tile([128, S], F32)
                        nc.sync.dma_start(out=rot[0:64, :], in_=qs[64:128, :])
                        nc.sync.dma_start(out=rot[64:128, :], in_=qs[0:64, :])
                        nc.vector.tensor_mul(out=qs[:], in0=qs[:], in1=cos_sb[:])
                        nc.vector.tensor_mul(out=rot[:], in0=rot[:], in1=sin_sb[:])
                        nc.vector.tensor_add(out=qs[:], in0=qs[:], in1=rot[:])
                        nc.sync.dma_start(out=dst[h], in_=qs[:])

        # ---------------- Phase 2: attention per head ----------------
        with (
            tc.tile_pool(name="qkv_pool", bufs=2) as qkv_pool,
            tc.tile_pool(name="et_pool", bufs=3) as et_pool,
            tc.tile_pool(name="psum_p2", bufs=1, space="PSUM") as psum_p2,
            tc.tile_pool(name="misc_pool", bufs=2) as misc_pool,
            tc.tile_pool(name="acc_pool", bufs=1) as acc_pool,
        ):
            ones_sb = acc_pool.tile([128, 1], F32)
            nc.vector.memset(ones_sb[:], 1.0)
            psum = acc_pool.tile([128, S], F32)  # denominator partial sums (SBUF)

            for h in range(HL):
                qt_sb = qkv_pool.tile([128, S], F32R, tag="qt")
                kt_sb = qkv_pool.tile([128, S], F32R, tag="kt")
                v_sb = qkv_pool.tile([128, TB, DH], F32R, tag="v")
                nc.sync.dma_start(out=qt_sb[:], in_=qt_d[h].bitcast(F32R))
                nc.sync.dma_start(out=kt_sb[:], in_=kt_d[h].bitcast(F32R))
                nc.sync.dma_start(
                    out=v_sb[:],
                    in_=v_d[:, :, h * DH : (h + 1) * DH]
                    .rearrange("t p m -> p t m")
                    .bitcast(F32R),
                )

                ctx_ps = psum_p2.tile([128, S], F32, tag="ctx", bufs=1)
                for tb in range(TB):
                    et = et_pool.tile([128, S], F32)
                    for n in range(NB):
                        sc = psum_p2.tile([128, 512], F32, tag="sc", bufs=4)
                        nc.tensor.matmul(
                            sc[:],
                            kt_sb[:, tb * 128 : (tb + 1) * 128],
                            qt_sb[:, n * 512 : (n + 1) * 512],
                            start=True,
                            stop=True,
                        )
                        nc.scalar.activation(
                            out=et[:, n * 512 : (n + 1) * 512].bitcast(F32R),
                            in_=sc[:],
                            func=mybir.ActivationFunctionType.Exp,
                        )
                    if tb == 0:
                        nc.vector.tensor_copy(out=psum[:], in_=et[:])
                    else:
                        nc.vector.tensor_add(out=psum[:], in0=psum[:], in1=et[:])
                    for n in range(NB):
                        nc.tensor.matmul(
                            ctx_ps[:, n * 512 : (n + 1) * 512],
                            v_sb[:, tb, :],
                            et[:, n * 512 : (n + 1) * 512].bitcast(F32R),
                            start=(tb == 0),
                            stop=(tb == TB - 1),
                        )

                # denominator: cross-partition sum of psum via ones-matmul
                den_sb = misc_pool.tile([1, S], F32, tag="den")
                for n in range(NB):
                    dps = psum_p2.tile([1, 512], F32, tag="sc", bufs=4)
                    nc.tensor.matmul(
                        dps[:],
                        ones_sb[:],
                        psum[:, n * 512 : (n + 1) * 512],
                        start=True,
                        stop=True,
                    )
                    nc.scalar.copy(
                        out=den_sb[:, n * 512 : (n + 1) * 512], in_=dps[:]
                    )
                # broadcast den across partitions via DRAM bounce
                # (partition-stride-0 APs are only legal on DRAM tensors)
                nc.sync.dma_start(out=den_d[h], in_=den_sb[:])
                bc = misc_pool.tile([128, S], F32, tag="bc")
                den_ap = den_d[h]
                bcast_src = bass.AP(
                    tensor=den_ap.tensor,
                    offset=den_ap.offset,
                    ap=[[0, 128]] + list(den_ap.ap[1:]),
                )
                nc.sync.dma_start(out=bc[:], in_=bcast_src)
                nc.vector.reciprocal(out=bc[:], in_=bc[:])
                ct_sb = misc_pool.tile([128, S], F32, tag="ct")
                nc.vector.tensor_mul(out=ct_sb[:], in0=ctx_ps[:], in1=bc[:])
                nc.sync.dma_start(out=ct_d[h], in_=ct_sb[:])

        # ---------------- Phase 3: output projection (partial) ----------------
        with (
            tc.tile_pool(name="wo_pool", bufs=1) as wo_pool,
            tc.tile_pool(name="ct_pool", bufs=2) as ct_pool,
            tc.tile_pool(name="out_pool", bufs=2) as out_pool,
            tc.tile_pool(name="psum_p3", bufs=8, space="PSUM") as psum_p3,
        ):
            wo_sb = wo_pool.tile([128, HL, D], F32R)
            nc.sync.dma_start(
                out=wo_sb[:], in_=wo_d[:].rearrange("c p m -> p c m").bitcast(F32R)
            )
            for m in range(TB):
                cts = ct_pool.tile([128, HL, 128], F32R)
                nc.sync.dma_start(
                    out=cts[:],
                    in_=ct_d[:, :, m * 128 : (m + 1) * 128]
                    .rearrange("c p m2 -> p c m2")
                    .bitcast(F32R),
                )
                osb = out_pool.tile([128, D], F32)
                for n in range(NB):
                    ops = psum_p3.tile([128, 512], F32)
                    for c in range(HL):
                        nc.tensor.matmul(
                            ops[:],
                            cts[:, c, :],
                            wo_sb[:, c, n * 512 : (n + 1) * 512],
                            start=(c == 0),
                            stop=(c == HL - 1),
                        )
                    nc.scalar.copy(out=osb[:, n * 512 : (n + 1) * 512], in_=ops[:])
                nc.sync.dma_start(out=out_d[m * 128 : (m + 1) * 128, :], in_=osb[:])

    nc.finalize()
    return nc


def _get_nc():
    if "nc" not in _NC_CACHE:
        _NC_CACHE["nc"] = build_nc()
    return _NC_CACHE["nc"]


def _rope_tables():
    inv_freq = 1.0 / (ROPE_THETA ** (np.arange(0, DH, 2, dtype=np.float32) / DH))
    freqs = np.arange(S, dtype=np.float32)[:, None] * inv_freq[None, :]
    emb = np.concatenate([freqs, freqs], axis=-1)  # [S, 128]
    cosT = np.ascontiguousarray(np.cos(emb).T.astype(np.float32))  # [128, S]
    sinT = np.cos(emb * 0)  # placeholder, replaced below
    sinT = np.sin(emb).T.astype(np.float32)
    sinS = sinT.copy()
    sinS[0:64, :] *= -1.0  # sign-folded rotate_half
    return cosT, np.ascontiguousarray(sinS)


def kernel(
    x, wq, bq, wk, bk, wv, bv, wo, bo, _trace=False, _tmpdir=None
):
    x = np.asarray(x, dtype=np.float32)
    wq = np.asarray(wq, dtype=np.float32)
    wk = np.asarray(wk, dtype=np.float32)
    wv = np.asarray(wv, dtype=np.float32)
    wo = np.asarray(wo, dtype=np.float32)
    bq = np.asarray(bq, dtype=np.float32)
    bk = np.asarray(bk, dtype=np.float32)
    bv = np.asarray(bv, dtype=np.float32)
    bo = np.asarray(bo, dtype=np.float32)

    nc = _get_nc()
    cosT, sinS = _rope_tables()

    # per-group weight packs
    def qk_pack(w, g):
        ws = w[g * 1024 : (g + 1) * 1024, :]  # [8*128 dout, D]
        # -> [h, ko, kp(d_in), m(d_out)]
        return np.ascontiguousarray(
            ws.reshape(HL, 128, KO, 128).transpose(0, 2, 3, 1)
        )

    packs = []
    for g in range(2):
        wv_s = wv[g * 1024 : (g + 1) * 1024, :]  # [1024 dout, D]
        wv_p = np.ascontiguousarray(wv_s.reshape(HL * DH, KO, 128).transpose(1, 2, 0))
        wo_s = wo[:, g * 1024 : (g + 1) * 1024]  # [D, 1024 d_in]
        wo_p = np.ascontiguousarray(wo_s.reshape(D, HL, 128).transpose(1, 2, 0))
        bq_p = np.ascontiguousarray(
            (bq[g * 1024 : (g + 1) * 1024] * QSCALE).reshape(HL, 128).T
        )
        bk_p = np.ascontiguousarray(bk[g * 1024 : (g + 1) * 1024].reshape(HL, 128).T)
        packs.append(
            dict(
                wq=qk_pack(wq, g),
                wk=qk_pack(wk, g),
                wv=wv_p,
                wo=wo_p,
                bq=bq_p,
                bk=bk_p,
            )
        )

    in_maps = []
    for c in range(8):
        b, g = c // 2, c % 2
        xt = np.ascontiguousarray(x[b].T).reshape(KO, 128, S)
        m = dict(packs[g])
        m["xt"] = xt
        m["cosT"] = cosT
        m["sinS"] = sinS
        in_maps.append(m)

    res = run_bass_kernel_spmd(
        nc,
        in_maps,
        core_ids=list(range(8)),
        trace=_trace,
        tmpdir=_tmpdir,
    )

    bo_eff = bo + wo @ bv
    out = np.empty((B, S, D), dtype=np.float32)
    for b in range(B):
        out[b] = res.results[2 * b]["out"] + res.results[2 * b + 1]["out"]
        out[b] += bo_eff[None, :]
    if _trace:
        kernel.last_result = res
    return out
